# revision 1
# baseline (speedup 1.0000x reference)
"""Trainium2 Bass kernel for CRF Viterbi decode (nn_CRFLayer).

Problem: emissions [512, 1024, 48] f32, mask [512,1024] (unused by reference),
transitions [48,48], start/end_transitions [48]. Output: best_paths [512, 1024]
int32 (Viterbi argmax decode, jax reference semantics: first-occurrence argmax).

Strategy (8 NeuronCores, pure data parallel over batch, 64 seqs/core):
  Forward (per core): scores s_t[b, j] kept in SBUF, batch on 64 partitions.
    Per step: cand[b,(j,i)] = s_{t-1}[b,i] + T[i,j]   (DVE tensor_tensor, bcast AP)
              pre[b,j]     = max_i cand                (DVE grouped tensor_reduce)
              s_t[b,j]     = pre + em_t[b,j]           (DVE tensor_add)
    Scores streamed to DRAM scratch in blocks (for exact backtrace recompute).
  Backtrace: tag_t = argmax_i(s_t[b,i] + T[i, tag_{t+1}]) recomputed per step.
    The per-batch transition-column gather T[:, tag_b] is done exactly with two
    tiny fp32 PE matmuls (broadcast tag via matmul against a static identity,
    one-hot compare on DVE, then one-hot @ T^T), then DVE max/max_index.
    All arithmetic is bit-exact vs the jax reference (single adds, exact max,
    first-occurrence argmax), so integer paths match exactly.
"""

import os
import sys
from contextlib import ExitStack

import numpy as np

sys.path.insert(0, "/opt/trn_rl_repo")

import concourse.bass as bass  # noqa: E402
import concourse.tile as tile  # noqa: E402
from concourse import bacc, mybir  # noqa: E402

F32 = mybir.dt.float32
U16 = mybir.dt.uint16
I32 = mybir.dt.int32

NUM_TAGS = 48
BATCH = 512
SEQ_LEN = 1024
N_CORES = 8
B_LOC = BATCH // N_CORES  # 64 sequences per core


def build_nc(
    S: int = SEQ_LEN,
    TB: int = 128,
    B: int = B_LOC,
    T: int = NUM_TAGS,
    fwd_only: bool = False,
    reps: int = 1,
    split: bool = True,
    hist_out: bool = False,
):
    """Build the per-core Bass program (same program on all cores, SPMD).

    split=True uses the 128-partition forward: current-tag axis j is halved
    across partition groups (p = g*64+b, j = 24*g + j_lo), so the two big DVE
    ops per step process 1152 elements instead of 2304. The new half-scores
    are reassembled into a replicated [128, 48] score tile with two aligned
    copies plus two cross-group stream_shuffles (verified on HW).
    """
    assert S % TB == 0
    nblk = S // TB
    H = T // 2  # 24, j-half width
    IDENT32 = list(range(32))

    nc = bacc.Bacc("TRN2", target_bir_lowering=False, debug=False, num_devices=N_CORES)

    em_d = nc.dram_tensor("emissions", [B, S, T], F32, kind="ExternalInput")
    trans_d = nc.dram_tensor("transitions", [T, T], F32, kind="ExternalInput")
    start_d = nc.dram_tensor("start_transitions", [T], F32, kind="ExternalInput")
    end_d = nc.dram_tensor("end_transitions", [T], F32, kind="ExternalInput")
    paths_d = nc.dram_tensor("paths", [B, S], I32, kind="ExternalOutput")
    hist_d = nc.dram_tensor(
        "hist", [B, S, T], F32, kind="ExternalOutput" if hist_out else "Internal"
    )  # scratch: forward scores

    with tile.TileContext(nc) as tc, ExitStack() as ctx:
        const = ctx.enter_context(tc.tile_pool(name="const", bufs=1))
        emp = ctx.enter_context(tc.tile_pool(name="emp", bufs=2))
        hip = ctx.enter_context(tc.tile_pool(name="hip", bufs=2))
        wrk = ctx.enter_context(tc.tile_pool(name="wrk", bufs=2))
        psum = ctx.enter_context(tc.tile_pool(name="psum", bufs=2, space="PSUM"))

        # ---- constants -------------------------------------------------
        # Trep[b, j, i] = T[i, j]  (j-major candidate layout)
        t_ap = trans_d.ap()  # [i, j]
        tt_flat = const.tile([1, T * T], F32)
        nc.sync.dma_start(
            tt_flat[:].rearrange("p (j i) -> p j i", j=T), t_ap.transpose([1, 0]).unsqueeze(0)
        )
        trep = None
        if not split:
            trep = const.tile([B, T, T], F32)
            nc.gpsimd.partition_broadcast(trep[:].rearrange("p j i -> p (j i)"), tt_flat[:])
        # T_T[j, i] = T[i, j] on 48 partitions (rhs of the gather matmul)
        t_t = const.tile([T, T], F32)
        nc.sync.dma_start(t_t[:], t_ap.transpose([1, 0]))
        # start/end broadcast over batch partitions
        start_b = None
        if not split:
            start_b = const.tile([B, T], F32)
            nc.sync.dma_start(start_b[:], start_d.ap().unsqueeze(0).broadcast_to([B, T]))
        end_b = const.tile([B, T], F32)
        nc.sync.dma_start(end_b[:], end_d.ap().unsqueeze(0).broadcast_to([B, T]))

        # diag01[b, b'] = 1.0 iff b == b' (identity, rhs of the tag-transpose mm)
        diag_i = const.tile([B, B], I32)
        nc.gpsimd.iota(diag_i[:], pattern=[[1, B]], base=0, channel_multiplier=-1)
        diag01 = const.tile([B, B], F32)
        nc.vector.tensor_scalar(diag01[:], diag_i[:], 0, None, op0=mybir.AluOpType.is_equal)
        # iota_part[j, b] = j  (partition index, f32, on 48 partitions)
        iota_p_i = const.tile([T, B], I32)
        nc.gpsimd.iota(iota_p_i[:], pattern=[[0, B]], base=0, channel_multiplier=1)
        iota_p = const.tile([T, B], F32)
        nc.vector.tensor_copy(iota_p[:], iota_p_i[:])

        # path8[b, t, 0:8]: max_index writes full 8-wide rows; col 0 is the tag
        path8 = const.tile([B, S, 8], U16)

        if split:
            # Trep2[g*64+b, j_lo, i] = T[i, 24g + j_lo]
            # NB: partition_broadcast ignores input free offsets on HW — each
            # source must sit at offset 0 of its own tile.
            tt_hi = const.tile([1, H * T], F32)
            nc.sync.dma_start(
                tt_hi[:].rearrange("p (j i) -> p j i", j=H),
                t_ap.transpose([1, 0])[H:T].unsqueeze(0),
            )
            # partition_broadcast also cannot write a partition-offset output
            # range on HW — broadcast at base 0, then stream_shuffle up.
            trep2 = const.tile([2 * B, H, T], F32)
            nc.gpsimd.partition_broadcast(
                trep2[0:B].rearrange("p j i -> p (j i)"), tt_flat[:, 0 : H * T]
            )
            tmp_hi = const.tile([B, H, T], F32)
            nc.gpsimd.partition_broadcast(tmp_hi[:].rearrange("p j i -> p (j i)"), tt_hi[:])
            nc.vector.stream_shuffle(
                trep2[B : 2 * B].rearrange("p j i -> p (j i)"),
                tmp_hi[:].rearrange("p j i -> p (j i)"),
                mask=IDENT32,
            )
            # start2[g*64+b, j_lo] = start[24g + j_lo]
            start2 = const.tile([2 * B, H], F32)
            nc.sync.dma_start(start2[0:B], start_d.ap()[0:H].unsqueeze(0).broadcast_to([B, H]))
            nc.sync.dma_start(
                start2[B : 2 * B], start_d.ap()[H:T].unsqueeze(0).broadcast_to([B, H])
            )

        def assemble_full(s_half):
            """[128, H] half-scores -> [128, T] replicated full scores."""
            sf = wrk.tile([2 * B, T], F32, tag="sfull")
            nc.vector.tensor_copy(sf[0:B, 0:H], s_half[0:B])
            nc.vector.tensor_copy(sf[B : 2 * B, H:T], s_half[B : 2 * B])
            nc.vector.stream_shuffle(sf[0:B, H:T], s_half[B : 2 * B], mask=IDENT32)
            nc.vector.stream_shuffle(sf[B : 2 * B, 0:H], s_half[0:B], mask=IDENT32)
            return sf

        for _rep in range(reps):
            # ---- forward ---------------------------------------------------
            em_tiles = []
            hist_tiles = []
            s_full = None
            for blk in range(nblk):
                if split:
                    em_t = emp.tile([2 * B, TB, H], F32, tag="em")
                    nc.sync.dma_start(em_t[0:B], em_d.ap()[:, blk * TB : (blk + 1) * TB, 0:H])
                    nc.sync.dma_start(
                        em_t[B : 2 * B], em_d.ap()[:, blk * TB : (blk + 1) * TB, H:T]
                    )
                    hist_t = hip.tile([2 * B, TB, H], F32, tag="hist")
                else:
                    em_t = emp.tile([B, TB, T], F32, tag="em")
                    nc.sync.dma_start(em_t[:], em_d.ap()[:, blk * TB : (blk + 1) * TB, :])
                    hist_t = hip.tile([B, TB, T], F32, tag="hist")
                em_tiles.append(em_t)
                hist_tiles.append(hist_t)

                for off in range(TB):
                    t = blk * TB + off
                    if split:
                        if t == 0:
                            nc.vector.tensor_add(hist_t[:, 0, :], start2[:], em_t[:, 0, :])
                        else:
                            cand = wrk.tile([2 * B, H, T], F32, tag="cand")
                            nc.vector.tensor_tensor(
                                cand[:],
                                s_full[:].unsqueeze(1).broadcast_to([2 * B, H, T]),
                                trep2[:],
                                op=mybir.AluOpType.add,
                            )
                            pre = wrk.tile([2 * B, H], F32, tag="pre")
                            nc.vector.tensor_reduce(
                                pre[:], cand[:], axis=mybir.AxisListType.X,
                                op=mybir.AluOpType.max,
                            )
                            nc.vector.tensor_add(hist_t[:, off, :], pre[:], em_t[:, off, :])
                        s_full = assemble_full(hist_t[:, off, :])
                        continue
                    if t == 0:
                        nc.vector.tensor_add(hist_t[:, 0, :], start_b[:], em_t[:, 0, :])
                        continue
                    s_prev = (
                        hist_t[:, off - 1, :]
                        if off > 0
                        else hist_tiles[blk - 1][:, TB - 1, :]
                    )
                    cand = wrk.tile([B, T, T], F32, tag="cand")
                    nc.vector.tensor_tensor(
                        cand[:],
                        s_prev.unsqueeze(1).broadcast_to([B, T, T]),
                        trep[:],
                        op=mybir.AluOpType.add,
                    )
                    pre = wrk.tile([B, T], F32, tag="pre")
                    nc.vector.tensor_reduce(
                        pre[:], cand[:], axis=mybir.AxisListType.X, op=mybir.AluOpType.max
                    )
                    nc.vector.tensor_add(hist_t[:, off, :], pre[:], em_t[:, off, :])

                if split:
                    nc.sync.dma_start(
                        hist_d.ap()[:, blk * TB : (blk + 1) * TB, 0:H], hist_t[0:B]
                    )
                    nc.sync.dma_start(
                        hist_d.ap()[:, blk * TB : (blk + 1) * TB, H:T], hist_t[B : 2 * B]
                    )
                else:
                    nc.sync.dma_start(hist_d.ap()[:, blk * TB : (blk + 1) * TB, :], hist_t[:])

            # ---- final argmax ----------------------------------------------
            fin = const.tile([B, T], F32)
            if split:
                nc.vector.tensor_add(fin[:], s_full[0:B, :], end_b[:])
            else:
                nc.vector.tensor_add(fin[:], hist_tiles[-1][:, TB - 1, :], end_b[:])
            m8f = const.tile([B, 8], F32)
            nc.vector.max(m8f[:], fin[:])
            nc.vector.max_index(path8[:, S - 1, :], m8f[:], fin[:])

            # ---- backtrace -------------------------------------------------
            for rblk in (range(nblk - 1, -1, -1) if not fwd_only else []):
                hr = hip.tile([B, TB, T], F32, tag="histr")
                nc.sync.dma_start(hr[:], hist_d.ap()[:, rblk * TB : (rblk + 1) * TB, :])
                for off in range(TB - 1, -1, -1):
                    t = rblk * TB + off
                    if t == S - 1:
                        continue
                    tag_f = wrk.tile([B, 1], F32, tag="tagf")
                    nc.vector.tensor_copy(tag_f[:], path8[:, t + 1, 0:1])
                    wrep = wrk.tile([B, T], F32, tag="wrep")
                    nc.vector.tensor_copy(wrep[:], tag_f[:].broadcast_to([B, T]))
                    tagb = psum.tile([T, B], F32, tag="tagb")
                    nc.tensor.matmul(tagb[:], wrep[:], diag01[:])
                    oht = wrk.tile([T, B], F32, tag="oht")
                    nc.vector.tensor_tensor(
                        oht[:], iota_p[:], tagb[:], op=mybir.AluOpType.is_equal
                    )
                    tcol = psum.tile([B, T], F32, tag="tcol")
                    nc.tensor.matmul(tcol[:], oht[:], t_t[:])
                    c48 = wrk.tile([B, T], F32, tag="c48")
                    nc.vector.tensor_add(c48[:], hr[:, off, :], tcol[:])
                    m8 = wrk.tile([B, 8], F32, tag="m8")
                    nc.vector.max(m8[:], c48[:])
                    nc.vector.max_index(path8[:, t, :], m8[:], c48[:])

            # ---- emit paths -------------------------------------------------
            paths_i = const.tile([B, S], I32)
            nc.vector.tensor_copy(paths_i[:], path8[:, :, 0])
            nc.sync.dma_start(paths_d.ap()[:], paths_i[:])

    nc.compile()
    return nc


def kernel(emissions, mask, transitions, start_transitions, end_transitions):
    """Full-input entry point: shards batch over 8 cores, runs SPMD, gathers."""
    from concourse.bass_utils import run_bass_kernel_spmd

    emissions = np.ascontiguousarray(np.asarray(emissions), dtype=np.float32)
    transitions = np.ascontiguousarray(np.asarray(transitions), dtype=np.float32)
    start_transitions = np.ascontiguousarray(np.asarray(start_transitions), dtype=np.float32)
    end_transitions = np.ascontiguousarray(np.asarray(end_transitions), dtype=np.float32)

    nc = build_nc()
    in_maps = []
    for c in range(N_CORES):
        sl = emissions[c * B_LOC : (c + 1) * B_LOC]
        in_maps.append(
            {
                "emissions": sl,
                "transitions": transitions,
                "start_transitions": start_transitions,
                "end_transitions": end_transitions,
            }
        )
    res = run_bass_kernel_spmd(nc, in_maps, list(range(N_CORES)))
    out = np.concatenate([r["paths"] for r in res.results], axis=0)
    return out.astype(np.int32)



# revision 2
# speedup vs baseline: 1.0998x; 1.0998x over previous
"""Trainium2 Bass kernel for CRF Viterbi decode (nn_CRFLayer).

Problem: emissions [512, 1024, 48] f32, mask [512,1024] (unused by reference),
transitions [48,48], start/end_transitions [48]. Output: best_paths [512, 1024]
int32 (Viterbi argmax decode, jax reference semantics: first-occurrence argmax).

Strategy (8 NeuronCores, pure data parallel over batch, 64 seqs/core):

Forward (per core): 128 partitions = (g, b) with g in {0,1} the j-half group,
b the sequence. Group g computes the 24 next-tags j in [24g, 24g+24).
Per step, 5 DVE ops:
  TT   cand[p, jl, i'] = s_full[p, i'] + trep2[p, jl, i']      (1152/partition)
  TR   pre[p, jl]      = max_i' cand                            (grouped max)
  TT   s_full[p, 0:24] = pre + em                               (em add, in place)
  2x stream_shuffle to exchange the halves across groups.
The g=1 replica of the transitions (trep2) is column-rotated by 24 so that each
group's own j-half lands at columns 0:24 of its s_full rows — this makes the
em-add a single 128-partition op with no per-group copies. s_full rows 0:64
hold the natural tag order and are streamed to DRAM as the score history.

Backtrace: tag_t = argmax_i(s_t[b,i] + T[i, tag_{t+1}]) recomputed per step
from the stored history. The transition-column gather T[:, tag_b] is exact via
two small fp32 PE matmuls (tag broadcast -> transpose against identity ->
one-hot compare -> one-hot @ T^T), then DVE add + max + max_index (first-
occurrence argmax, matching jnp.argmax). All arithmetic replicates the
reference's f32 ops bit-exactly, so the integer paths match exactly.

All derived constants (trep2, start2, T^T, identity, iota) are precomputed on
the host in kernel() and passed as extra inputs.
"""

import sys
from contextlib import ExitStack

import numpy as np

sys.path.insert(0, "/opt/trn_rl_repo")

import concourse.bass as bass  # noqa: E402
import concourse.tile as tile  # noqa: E402
from concourse import bacc, mybir  # noqa: E402

F32 = mybir.dt.float32
U16 = mybir.dt.uint16
I32 = mybir.dt.int32

NUM_TAGS = 48
BATCH = 512
SEQ_LEN = 1024
N_CORES = 8
B_LOC = BATCH // N_CORES  # 64 sequences per core
H = NUM_TAGS // 2  # 24
IDENT32 = list(range(32))


def build_nc(S: int = SEQ_LEN, TB: int = 128, B: int = B_LOC, T: int = NUM_TAGS):
    """Build the per-core Bass program (same program on all cores, SPMD)."""
    assert S % TB == 0
    nblk = S // TB
    P = 2 * B  # 128 partitions

    nc = bacc.Bacc("TRN2", target_bir_lowering=False, debug=False, num_devices=N_CORES)

    em_d = nc.dram_tensor("emissions", [B, S, T], F32, kind="ExternalInput")
    trep2_d = nc.dram_tensor("trep2", [P, H, T], F32, kind="ExternalInput")
    start2_d = nc.dram_tensor("start2", [P, H], F32, kind="ExternalInput")
    endrow_d = nc.dram_tensor("endrow", [B, T], F32, kind="ExternalInput")
    tt_d = nc.dram_tensor("t_t", [T, T], F32, kind="ExternalInput")
    diag_d = nc.dram_tensor("diag01", [B, B], F32, kind="ExternalInput")
    iotap_d = nc.dram_tensor("iota_p", [T, B], F32, kind="ExternalInput")
    paths_d = nc.dram_tensor("paths", [B, S], I32, kind="ExternalOutput")
    hist_d = nc.dram_tensor("hist", [B, S, T], F32, kind="Internal")

    with tile.TileContext(nc) as tc, ExitStack() as ctx:
        const = ctx.enter_context(tc.tile_pool(name="const", bufs=1))
        emp = ctx.enter_context(tc.tile_pool(name="emp", bufs=2))
        hip = ctx.enter_context(tc.tile_pool(name="hip", bufs=2))
        hrp = ctx.enter_context(tc.tile_pool(name="hrp", bufs=2))
        wrk = ctx.enter_context(tc.tile_pool(name="wrk", bufs=3))
        psum = ctx.enter_context(tc.tile_pool(name="psum", bufs=2, space="PSUM"))

        # ---- constants (all host-precomputed) ---------------------------
        trep2 = const.tile([P, H, T], F32, name="trep2")
        nc.sync.dma_start(trep2[:], trep2_d.ap())
        start2 = const.tile([P, H], F32, name="start2")
        nc.sync.dma_start(start2[:], start2_d.ap())
        end_b = const.tile([B, T], F32, name="end_b")
        nc.sync.dma_start(end_b[:], endrow_d.ap())
        t_t = const.tile([T, T], F32, name="t_t")
        nc.sync.dma_start(t_t[:], tt_d.ap())
        diag01 = const.tile([B, B], F32, name="diag01")
        nc.sync.dma_start(diag01[:], diag_d.ap())
        iota_p = const.tile([T, B], F32, name="iota_p")
        nc.sync.dma_start(iota_p[:], iotap_d.ap())

        # path8[b, t, 0:8]: max_index writes 8-wide rows; col 0 is the tag
        path8 = const.tile([B, S, 8], U16, name="path8")

        # ---- forward ----------------------------------------------------
        hist_prev = None
        hist_t = None
        for blk in range(nblk):
            em_t = emp.tile([P, TB, H], F32, tag="em")
            nc.sync.dma_start(em_t[0:B], em_d.ap()[:, blk * TB : (blk + 1) * TB, 0:H])
            nc.sync.dma_start(
                em_t[B:P], em_d.ap()[:, blk * TB : (blk + 1) * TB, H:T]
            )
            hist_prev = hist_t
            hist_t = hip.tile([P, TB, T], F32, tag="hist")

            for off in range(TB):
                t = blk * TB + off
                if t == 0:
                    nc.vector.tensor_tensor(
                        hist_t[:, 0, 0:H], start2[:], em_t[:, 0, :],
                        op=mybir.AluOpType.add,
                    )
                else:
                    s_prev = (
                        hist_t[:, off - 1, :] if off > 0
                        else hist_prev[:, TB - 1, :]
                    )
                    cand = wrk.tile([P, H, T], F32, tag="cand")
                    nc.vector.tensor_tensor(
                        cand[:],
                        s_prev.unsqueeze(1).broadcast_to([P, H, T]),
                        trep2[:],
                        op=mybir.AluOpType.add,
                    )
                    pre = wrk.tile([P, H], F32, tag="pre")
                    nc.vector.tensor_reduce(
                        pre[:], cand[:], axis=mybir.AxisListType.X,
                        op=mybir.AluOpType.max,
                    )
                    nc.vector.tensor_tensor(
                        hist_t[:, off, 0:H], pre[:], em_t[:, off, :],
                        op=mybir.AluOpType.add,
                    )
                # exchange halves: each group's own half sits at cols 0:24
                nc.vector.stream_shuffle(
                    hist_t[0:B, off, H:T], hist_t[B:P, off, 0:H], mask=IDENT32
                )
                nc.vector.stream_shuffle(
                    hist_t[B:P, off, H:T], hist_t[0:B, off, 0:H], mask=IDENT32
                )

            nc.sync.dma_start(
                hist_d.ap()[:, blk * TB : (blk + 1) * TB, :], hist_t[0:B]
            )

        # ---- final argmax ----------------------------------------------
        fin = const.tile([B, T], F32, name="fin")
        nc.vector.tensor_tensor(
            fin[:], hist_t[0:B, TB - 1, :], end_b[:], op=mybir.AluOpType.add
        )
        m8f = const.tile([B, 8], F32, name="m8f")
        nc.vector.max(m8f[:], fin[:])
        nc.vector.max_index(path8[:, S - 1, :], m8f[:], fin[:])

        # ---- backtrace --------------------------------------------------
        for rblk in range(nblk - 1, -1, -1):
            hr = hrp.tile([B, TB, T], F32, tag="hr")
            nc.sync.dma_start(hr[:], hist_d.ap()[:, rblk * TB : (rblk + 1) * TB, :])
            for off in range(TB - 1, -1, -1):
                t = rblk * TB + off
                if t == S - 1:
                    continue
                # one-hot of tag_{t+1}: broadcast-cast, transpose via PE,
                # compare against the partition iota
                wrep = wrk.tile([B, T], F32, tag="wrep")
                nc.vector.tensor_copy(
                    wrep[:], path8[:, t + 1, 0:1].broadcast_to([B, T])
                )
                tagb = psum.tile([T, B], F32, tag="tagb")
                nc.tensor.matmul(tagb[:], wrep[:], diag01[:])
                oht = wrk.tile([T, B], F32, tag="oht")
                nc.vector.tensor_tensor(
                    oht[:], iota_p[:], tagb[:], op=mybir.AluOpType.is_equal
                )
                tcol = psum.tile([B, T], F32, tag="tcol")
                nc.tensor.matmul(tcol[:], oht[:], t_t[:])
                c48 = wrk.tile([B, T], F32, tag="c48")
                nc.vector.tensor_tensor(
                    c48[:], hr[:, off, :], tcol[:], op=mybir.AluOpType.add
                )
                m8 = wrk.tile([B, 8], F32, tag="m8")
                nc.vector.max(m8[:], c48[:])
                nc.vector.max_index(path8[:, t, :], m8[:], c48[:])

        # ---- emit paths -------------------------------------------------
        paths_i = const.tile([B, S], I32, name="paths_i")
        nc.vector.tensor_copy(paths_i[:], path8[:, :, 0])
        nc.sync.dma_start(paths_d.ap()[:], paths_i[:])

    nc.compile()
    return nc


def make_derived(transitions, start_transitions, end_transitions):
    """Host-precomputed derived constant tensors (per-core replicated)."""
    T = NUM_TAGS
    B = B_LOC
    Tm = np.ascontiguousarray(transitions, dtype=np.float32)
    # trep2[g*64+b, jl, i']: g0: T[i', jl]; g1: T[(i'+24)%48, 24+jl]
    g0 = Tm.T[0:H, :]  # [jl, i] = T[i, jl]
    rot = np.roll(np.arange(T), -H)  # i' -> (i'+24)%48
    g1 = Tm.T[H:T, :][:, rot]  # [jl, i'] = T[(i'+24)%48, 24+jl]
    trep2 = np.empty((2 * B, H, T), dtype=np.float32)
    trep2[0:B] = g0[None, :, :]
    trep2[B:] = g1[None, :, :]
    start2 = np.empty((2 * B, H), dtype=np.float32)
    start2[0:B] = np.asarray(start_transitions, dtype=np.float32)[None, 0:H]
    start2[B:] = np.asarray(start_transitions, dtype=np.float32)[None, H:T]
    endrow = np.broadcast_to(
        np.asarray(end_transitions, dtype=np.float32)[None, :], (B, T)
    ).copy()
    t_t = np.ascontiguousarray(Tm.T)
    diag01 = np.eye(B, dtype=np.float32)
    iota_p = np.broadcast_to(
        np.arange(T, dtype=np.float32)[:, None], (T, B)
    ).copy()
    return {
        "trep2": trep2,
        "start2": start2,
        "endrow": endrow,
        "t_t": t_t,
        "diag01": diag01,
        "iota_p": iota_p,
    }


def make_in_maps(inputs):
    """Shard full inputs into per-core input maps."""
    emissions = np.ascontiguousarray(np.asarray(inputs["emissions"]), dtype=np.float32)
    derived = make_derived(
        np.asarray(inputs["transitions"]),
        np.asarray(inputs["start_transitions"]),
        np.asarray(inputs["end_transitions"]),
    )
    in_maps = []
    for c in range(N_CORES):
        m = {"emissions": emissions[c * B_LOC : (c + 1) * B_LOC]}
        m.update(derived)
        in_maps.append(m)
    return in_maps


def kernel(emissions, mask, transitions, start_transitions, end_transitions):
    """Full-input entry point: shards batch over 8 cores, runs SPMD, gathers."""
    from concourse.bass_utils import run_bass_kernel_spmd

    nc = build_nc()
    in_maps = make_in_maps(
        {
            "emissions": emissions,
            "transitions": transitions,
            "start_transitions": start_transitions,
            "end_transitions": end_transitions,
        }
    )
    res = run_bass_kernel_spmd(nc, in_maps, list(range(N_CORES)))
    out = np.concatenate([np.asarray(r["paths"]) for r in res.results], axis=0)
    return out.astype(np.int32)
